# revision 24
# baseline (speedup 1.0000x reference)
"""Trainium2 Bass kernel for nn_ContrastiveLoss (wav2vec2-style contrastive loss).

Shapes (hardcoded): B=8, C=256, T=1024, M=512 masked positions, K=100 negatives.
Sharding: pure data parallel — batch row b -> NeuronCore b (8 cores).

Work per (m, k): dot[m,k] = sum_c neg[m,k,c]*ctx[m,c]; sumsq[m,k] = sum_c neg^2.
Then cosine-normalize, logsumexp over K+1 logits, per-row loss; host sums.

Engine assignment:
- TensorE computes all 51,200 dots as batched mat-vecs: per (m, c-chunk) one
  matmul with stationary = negT[m] chunk [128c, 128k-cols] (fp8) and moving =
  ctx[m] chunk [128c, 1]; PSUM accumulates the two chunks. Results land as
  dots^T [k, m] per 128-m group, copied to SBUF and transposed back to
  [m, k] with a PE transpose.
- ScalarE squares the fp8 negatives stream in big [128, KCH*C] ACTIVATE ops
  (no accumulator, output bf16); two tiles ship as bf16 and are squared on
  VectorE instead (tensor_tensor mult, 2x mode) to balance the engines.
- VectorE halves the squares five times with 2x bf16 tensor_tensor adds,
  then one 1x tensor_reduce per tile yields sumsq[m,k]. (All DVE ops with
  accum_out are hard-capped at 1x rate, so per-k accumulate ops lose.)

Streams (host-prepped; conversion/layout on host):
- nega  [M, K, C] fp8e4  (13.1 MB) - sumsq stream
- negab [2, 128, KCH, C] bf16      - the two VectorE-squared tiles
- negt  [2, 128, M*K+PAD] fp8 (13.1 MB) - c-major transposed negatives for PE
- ctxt  [2, 128, M] fp8  - PE moving operands
- ctxg/posg [M, C] bf16  - pos-similarity + ctx/pos norms
"""

import numpy as np

TEMP = 0.1
EPS = 1e-8
B, C, T = 8, 256, 1024
M = 512  # masked positions per batch row
K = 100  # negatives per masked position
P = 128  # partitions
G = M // P  # m-groups per core (4)
KCH = 25  # k's per streamed fp8 tile: [128, KCH, C] fp8 = 819 KB
NKC = K // KCH  # fp8 stream tiles per m-group (4)
MB = 64  # m's per negt (PE) tile -> [128, MB*K+PAD] bf16 = 1.65 MB
PAD = 28  # lhsT 128-col slices overrun the last m's 100 cols by 28
NMB = P // MB  # negt tiles per chunk per m-group (2)
# (group, tile) pairs whose squares run on VectorE from a bf16 copy
# (tensor_tensor mult gets the 2x bf16 perf mode; relieves ScalarE).
BF_TILES = ((1, 2), (3, 1))

_NC = None
def _build_nc():
    import concourse.bacc as bacc
    import concourse.tile as tile
    from concourse import masks, mybir

    f32 = mybir.dt.float32
    bf16 = mybir.dt.bfloat16
    fp8 = mybir.dt.float8e4
    Alu = mybir.AluOpType
    Act = mybir.ActivationFunctionType

    nc = bacc.Bacc(trn_type="TRN2")
    nega = nc.dram_tensor("nega", [M, K, C], fp8, kind="ExternalInput")
    negab = nc.dram_tensor(
        "negab", [len(BF_TILES), P, KCH, C], bf16, kind="ExternalInput"
    )
    negt = nc.dram_tensor("negt", [2, P, M * K + PAD], fp8, kind="ExternalInput")
    ctxt = nc.dram_tensor("ctxt", [2, P, M], fp8, kind="ExternalInput")
    ctxg = nc.dram_tensor("ctxg", [M, C], bf16, kind="ExternalInput")
    posg = nc.dram_tensor("posg", [M, C], bf16, kind="ExternalInput")
    rowloss = nc.dram_tensor("rowloss", [P, G], f32, kind="ExternalOutput")

    from contextlib import ExitStack

    with tile.TileContext(nc) as tc, ExitStack() as es:
        pool_specs = dict(
            single=1, astream=3, sqp=4, fold1=2, fold2=2, fold3=2, fold4=2,
            fold5=2, ntp0=2, ntp1=2, grp=2, pg=G, scrp=2, sdtp=2, outp=1,
        )
        pools = {
            n: es.enter_context(tc.tile_pool(name=n, bufs=b))
            for n, b in pool_specs.items()
        }
        (single, astream, sqp, fold1p, fold2p, fold3p, fold4p, fold5p, ntp0,
         ntp1, grp, pg, scrp, sdtp, outp) = (
            pools[n] for n in (
                "single", "astream", "sqp", "fold1", "fold2", "fold3", "fold4",
                "fold5", "ntp0", "ntp1", "grp", "pg", "scrp", "sdtp", "outp",
            )
        )
        pdtp = es.enter_context(tc.psum_pool(name="pdt", bufs=2))
        pdotsp = es.enter_context(tc.psum_pool(name="pdots", bufs=G))
        if True:
            out_t = outp.tile([P, G], f32)
            identity = single.tile([P, P], f32)
            masks.make_identity(nc, identity[:])
            dummy = single.tile([P, C], bf16)

            ctxt_s = {}
            for ch in range(2):
                ctxt_s[ch] = single.tile([P, M], fp8, name=f"ctxt{ch}")
                nc.sync.dma_start(out=ctxt_s[ch][:], in_=ctxt[ch])

            css_a = single.tile([P, G], f32)
            pss_a = single.tile([P, G], f32)
            cpd_a = single.tile([P, G], f32)
            crn_a = single.tile([P, G], f32)
            prn_a = single.tile([P, G], f32)
            se_a = single.tile([P, G], f32)
            lnse_a = single.tile([P, G], f32)
            gt = {}
            for g in range(G):
                gt[g] = dict(
                    negss=pg.tile([P, K], f32, tag="negss", name=f"negss{g}"),
                    logits=pg.tile([P, K + 1], f32, tag="logits", name=f"logits{g}"),
                    pdots=pdotsp.tile([P, P], f32, tag="pdots", name=f"pdots{g}"),
                )

            for g in range(G):
                m0 = g * P
                d = gt[g]
                ctx_t = grp.tile([P, C], bf16, tag="ctx")
                pos_t = grp.tile([P, C], bf16, tag="pos")
                nc.sync.dma_start(out=ctx_t[:], in_=ctxg[m0 : m0 + P, :])
                nc.sync.dma_start(out=pos_t[:], in_=posg[m0 : m0 + P, :])

                nc.vector.scalar_tensor_tensor(
                    out=dummy[:], in0=ctx_t[:], scalar=1.0, in1=ctx_t[:],
                    op0=Alu.mult, op1=Alu.mult, accum_out=css_a[:, g : g + 1],
                )
                nc.vector.scalar_tensor_tensor(
                    out=dummy[:], in0=pos_t[:], scalar=1.0, in1=pos_t[:],
                    op0=Alu.mult, op1=Alu.mult, accum_out=pss_a[:, g : g + 1],
                )
                nc.vector.scalar_tensor_tensor(
                    out=dummy[:], in0=ctx_t[:], scalar=1.0, in1=pos_t[:],
                    op0=Alu.mult, op1=Alu.mult, accum_out=cpd_a[:, g : g + 1],
                )

                # ---- sumsq: ScalarE squares (big op), VectorE folds + 3D reduce ----
                for t in range(NKC):
                    sq = sqp.tile([P, KCH, C], bf16, tag="sq")
                    if (g, t) in BF_TILES:
                        bidx = BF_TILES.index((g, t))
                        ntb = astream.tile([P, KCH, C], bf16, tag="ntb", name="ntb")
                        nc.sync.dma_start(out=ntb[:], in_=negab[bidx])
                        nc.vector.tensor_tensor(
                            out=sq[:], in0=ntb[:], in1=ntb[:], op=Alu.mult
                        )
                    else:
                        nt = astream.tile([P, KCH, C], fp8, tag="nt")
                        nc.sync.dma_start(
                            out=nt[:],
                            in_=nega[m0 : m0 + P, t * KCH : (t + 1) * KCH, :],
                        )
                        nc.scalar.activation(out=sq[:], in_=nt[:], func=Act.Square)
                    f1 = fold1p.tile([P, KCH, C // 2], bf16, tag="f1")
                    nc.vector.tensor_tensor(
                        out=f1[:], in0=sq[:, :, 0 : C // 2],
                        in1=sq[:, :, C // 2 : C], op=Alu.add,
                    )
                    f2 = fold2p.tile([P, KCH, C // 4], bf16, tag="f2")
                    nc.vector.tensor_tensor(
                        out=f2[:], in0=f1[:, :, 0 : C // 4],
                        in1=f1[:, :, C // 4 : C // 2], op=Alu.add,
                    )
                    f3 = fold3p.tile([P, KCH, C // 8], bf16, tag="f3")
                    nc.vector.tensor_tensor(
                        out=f3[:], in0=f2[:, :, 0 : C // 8],
                        in1=f2[:, :, C // 8 : C // 4], op=Alu.add,
                    )
                    f4 = fold4p.tile([P, KCH, C // 16], bf16, tag="f4")
                    nc.vector.tensor_tensor(
                        out=f4[:], in0=f3[:, :, 0 : C // 16],
                        in1=f3[:, :, C // 16 : C // 8], op=Alu.add,
                    )
                    f5 = fold5p.tile([P, KCH, C // 32], bf16, tag="f5")
                    nc.vector.tensor_tensor(
                        out=f5[:], in0=f4[:, :, 0 : C // 32],
                        in1=f4[:, :, C // 32 : C // 16], op=Alu.add,
                    )
                    nc.vector.tensor_reduce(
                        out=d["negss"][:, t * KCH : (t + 1) * KCH],
                        in_=f5[:], axis=mybir.AxisListType.X, op=Alu.add,
                    )

                # ---- dots: TensorE batched mat-vecs into pdt (dots^T) ----
                pdt = pdtp.tile([P, P], f32, tag="pdt")
                for half in range(NMB):
                    mh = m0 + half * MB
                    nts = {}
                    for ch, pool in ((0, ntp0), (1, ntp1)):
                        nts[ch] = pool.tile(
                            [P, MB * K + PAD], fp8, tag=f"nt{ch}", name=f"ntt{ch}"
                        )
                        nc.sync.dma_start(
                            out=nts[ch][:],
                            in_=negt[ch, :, mh * K : mh * K + MB * K + PAD],
                        )
                    for i in range(MB):
                        mcol = half * MB + i
                        mg = m0 + mcol
                        for ch in range(2):
                            nc.tensor.matmul(
                                out=pdt[:, mcol : mcol + 1],
                                lhsT=nts[ch][:, i * K : i * K + P],
                                rhs=ctxt_s[ch][:, mg : mg + 1],
                                start=(ch == 0),
                                stop=(ch == 1),
                            )
                # dots^T [k, m] -> SBUF -> PE transpose -> pdots [m, k]
                sdt = sdtp.tile([P, P], f32, tag="sdt")
                nc.vector.tensor_copy(sdt[:], pdt[:])
                nc.tensor.transpose(d["pdots"][:], sdt[:], identity[:])

            # ---- batched epilogue, grouped by ACT function ----
            nrn = {}
            for g in range(G):
                nrn[g] = pg.tile([P, K], f32, tag="nrn", name=f"nrn{g}")
            nc.scalar.sqrt(css_a[:], css_a[:])
            nc.scalar.sqrt(pss_a[:], pss_a[:])
            for g in range(G):
                nc.scalar.sqrt(gt[g]["negss"][:], gt[g]["negss"][:])
            nc.vector.tensor_scalar_max(css_a[:], css_a[:], EPS)
            nc.vector.tensor_scalar_max(pss_a[:], pss_a[:], EPS)
            nc.vector.reciprocal(crn_a[:], css_a[:])
            nc.vector.reciprocal(prn_a[:], pss_a[:])
            for g in range(G):
                d = gt[g]
                nc.vector.tensor_scalar_max(d["negss"][:], d["negss"][:], EPS)
                nc.vector.reciprocal(nrn[g][:], d["negss"][:])
                # logits: col 0 = positive sim, cols 1..K = negative sims
                nc.vector.scalar_tensor_tensor(
                    out=d["logits"][:, 0:1], in0=cpd_a[:, g : g + 1],
                    scalar=crn_a[:, g : g + 1], in1=prn_a[:, g : g + 1],
                    op0=Alu.mult, op1=Alu.mult,
                )
                nc.vector.scalar_tensor_tensor(
                    out=d["logits"][:, 1 : K + 1], in0=d["pdots"][:, 0:K],
                    scalar=crn_a[:, g : g + 1], in1=nrn[g][:],
                    op0=Alu.mult, op1=Alu.mult,
                )
            # |logits| <= 1 so exp(logit/TEMP) <= e^10 — no max-shift needed
            for g in range(G):
                d = gt[g]
                esc = scrp.tile([P, K + 1], f32, tag="esc")
                nc.scalar.activation(
                    out=esc[:], in_=d["logits"][:], func=Act.Exp,
                    scale=1.0 / TEMP, accum_out=se_a[:, g : g + 1],
                )
            nc.scalar.activation(out=lnse_a[:], in_=se_a[:], func=Act.Ln)
            for g in range(G):
                d = gt[g]
                nc.vector.scalar_tensor_tensor(
                    out=out_t[:, g : g + 1], in0=d["logits"][:, 0:1],
                    scalar=-1.0 / TEMP, in1=lnse_a[:, g : g + 1],
                    op0=Alu.mult, op1=Alu.add,
                )
            nc.sync.dma_start(out=rowloss[:], in_=out_t[:])
    nc.finalize()
    return nc


def _get_nc():
    global _NC
    if _NC is None:
        _NC = _build_nc()
    return _NC


def make_in_maps(context, positive, negatives, mask_indices):
    import ml_dtypes

    bf = ml_dtypes.bfloat16
    f8 = ml_dtypes.float8_e4m3
    context = np.asarray(context, dtype=np.float32)
    positive = np.asarray(positive, dtype=np.float32)
    negatives = np.asarray(negatives, dtype=np.float32)
    mask = np.asarray(mask_indices).astype(bool)
    in_maps = []
    for b in range(B):
        idx = np.flatnonzero(mask[b])
        assert idx.size == M, f"row {b}: expected {M} masked, got {idx.size}"
        ctx_m = np.ascontiguousarray(context[b].T[idx])  # [M, C] f32
        pos_m = np.ascontiguousarray(positive[b].T[idx])  # [M, C] f32
        neg = negatives[b]  # [M, K, C] f32
        # c-major transposed negatives for the PE, flat (m,k) + PAD zeros
        negT = neg.transpose(2, 0, 1).reshape(C, M * K).astype(f8)  # [C, M*K]
        negt = np.zeros((2, P, M * K + PAD), dtype=f8)
        negt[0, :, : M * K] = negT[:P]
        negt[1, :, : M * K] = negT[P:]
        ctxT = ctx_m.T.astype(f8)  # [C, M]
        ctxt = np.stack([ctxT[:P], ctxT[P:]])  # [2, 128, M]
        negab = np.stack(
            [
                neg[g * P : (g + 1) * P, t * KCH : (t + 1) * KCH, :].astype(bf)
                for (g, t) in BF_TILES
            ]
        )
        in_maps.append(
            {
                "nega": neg.astype(f8),
                "negab": negab,
                "negt": negt,
                "ctxt": np.ascontiguousarray(ctxt),
                "ctxg": ctx_m.astype(bf),
                "posg": pos_m.astype(bf),
            }
        )
    return in_maps


def kernel(context, positive, negatives, mask_indices, num_masked):
    from concourse.bass_utils import run_bass_kernel_spmd

    nm = int(np.asarray(num_masked))
    assert nm == M, f"kernel hardcodes num_masked={M}, got {nm}"
    assert np.asarray(context).shape == (B, C, T)
    assert np.asarray(negatives).shape == (B, M, K, C)

    in_maps = make_in_maps(context, positive, negatives, mask_indices)
    res = run_bass_kernel_spmd(_get_nc(), in_maps, core_ids=list(range(B)))
    total = np.float64(0.0)
    for r in res.results:
        total += r["rowloss"].astype(np.float64).sum()
    return np.float32(total / (B * M))


# revision 26
# speedup vs baseline: 1.0236x; 1.0236x over previous
"""Trainium2 Bass kernel for nn_ContrastiveLoss (wav2vec2-style contrastive loss).

Shapes (hardcoded): B=8, C=256, T=1024, M=512 masked positions, K=100 negatives.
Sharding: pure data parallel — batch row b -> NeuronCore b (8 cores).

Work per (m, k): dot[m,k] = sum_c neg[m,k,c]*ctx[m,c]; sumsq[m,k] = sum_c neg^2.
Then cosine-normalize, logsumexp over K+1 logits, per-row loss; host sums.

Engine assignment:
- TensorE computes all 51,200 dots as batched mat-vecs: per (m, c-chunk) one
  matmul with stationary = negT[m] chunk [128c, 128k-cols] (fp8) and moving =
  ctx[m] chunk [128c, 1]; PSUM accumulates the two chunks. Results land as
  dots^T [k, m] per 128-m group, copied to SBUF and transposed back to
  [m, k] with a PE transpose.
- ScalarE squares the fp8 negatives stream in big [128, KCH*C] ACTIVATE ops
  (no accumulator, output bf16); two tiles ship as bf16 and are squared on
  VectorE instead (tensor_tensor mult, 2x mode) to balance the engines.
- VectorE halves the squares five times with 2x bf16 tensor_tensor adds,
  then one 1x tensor_reduce per tile yields sumsq[m,k]. (All DVE ops with
  accum_out are hard-capped at 1x rate, so per-k accumulate ops lose.)

Streams (host-prepped; conversion/layout on host):
- nega  [M, K, C] fp8e4  (13.1 MB) - sumsq stream
- negab [2, 128, KCH, C] bf16      - the two VectorE-squared tiles
- negt  [2, 128, M*K+PAD] fp8 (13.1 MB) - c-major transposed negatives for PE
- ctxt  [2, 128, M] fp8  - PE moving operands
- ctxg/posg [M, C] bf16  - pos-similarity + ctx/pos norms
"""

import numpy as np

TEMP = 0.1
EPS = 1e-8
B, C, T = 8, 256, 1024
M = 512  # masked positions per batch row
K = 100  # negatives per masked position
P = 128  # partitions
G = M // P  # m-groups per core (4)
KCH = 25  # k's per streamed fp8 tile: [128, KCH, C] fp8 = 819 KB
NKC = K // KCH  # fp8 stream tiles per m-group (4)
MB = 64  # m's per negt (PE) tile -> [128, MB*K+PAD] bf16 = 1.65 MB
PAD = 28  # lhsT 128-col slices overrun the last m's 100 cols by 28
NMB = P // MB  # negt tiles per chunk per m-group (2)
# (group, tile) pairs whose squares run on VectorE from a bf16 copy
# (tensor_tensor mult gets the 2x bf16 perf mode; relieves ScalarE).
BF_TILES = ((1, 2), (3, 1))

_NC = None
def _build_nc():
    import concourse.bacc as bacc
    import concourse.tile as tile
    from concourse import masks, mybir

    f32 = mybir.dt.float32
    bf16 = mybir.dt.bfloat16
    fp8 = mybir.dt.float8e4
    Alu = mybir.AluOpType
    Act = mybir.ActivationFunctionType

    nc = bacc.Bacc(trn_type="TRN2")
    nega = nc.dram_tensor("nega", [M, K, C], fp8, kind="ExternalInput")
    negab = nc.dram_tensor(
        "negab", [len(BF_TILES), P, KCH, C], bf16, kind="ExternalInput"
    )
    negt = nc.dram_tensor("negt", [2, P, M * K + PAD], fp8, kind="ExternalInput")
    ctxt = nc.dram_tensor("ctxt", [2, P, M], fp8, kind="ExternalInput")
    ctxg = nc.dram_tensor("ctxg", [M, C], bf16, kind="ExternalInput")
    posg = nc.dram_tensor("posg", [M, C], bf16, kind="ExternalInput")
    rowloss = nc.dram_tensor("rowloss", [P, G], f32, kind="ExternalOutput")

    from contextlib import ExitStack

    with tile.TileContext(nc) as tc, ExitStack() as es:
        pool_specs = dict(
            single=1, astream=3, sqp=4, fold1=2, fold2=2, fold3=2, fold4=2,
            fold5=2, ntp0=2, ntp1=2, grp=2, pg=G, scrp=2, sdtp=2, outp=1,
        )
        pools = {
            n: es.enter_context(tc.tile_pool(name=n, bufs=b))
            for n, b in pool_specs.items()
        }
        (single, astream, sqp, fold1p, fold2p, fold3p, fold4p, fold5p, ntp0,
         ntp1, grp, pg, scrp, sdtp, outp) = (
            pools[n] for n in (
                "single", "astream", "sqp", "fold1", "fold2", "fold3", "fold4",
                "fold5", "ntp0", "ntp1", "grp", "pg", "scrp", "sdtp", "outp",
            )
        )
        pdtp = es.enter_context(tc.psum_pool(name="pdt", bufs=2))
        pdotsp = es.enter_context(tc.psum_pool(name="pdots", bufs=G))
        if True:
            out_t = outp.tile([P, G], f32)
            identity = single.tile([P, P], f32)
            masks.make_identity(nc, identity[:])
            dummy = single.tile([P, C], bf16)

            # prefetch the first two fp8 A-tiles so ScalarE starts early
            nt_pre = {}
            for t in range(2):
                nt_pre[t] = astream.tile(
                    [P, KCH, C], fp8, tag="nt", name=f"ntpre{t}"
                )
                nc.sync.dma_start(
                    out=nt_pre[t][:], in_=nega[0:P, t * KCH : (t + 1) * KCH, :]
                )

            ctxt_s = {}
            for ch in range(2):
                ctxt_s[ch] = single.tile([P, M], fp8, name=f"ctxt{ch}")
                nc.sync.dma_start(out=ctxt_s[ch][:], in_=ctxt[ch])

            css_a = single.tile([P, G], f32)
            pss_a = single.tile([P, G], f32)
            cpd_a = single.tile([P, G], f32)
            crn_a = single.tile([P, G], f32)
            prn_a = single.tile([P, G], f32)
            se_a = single.tile([P, G], f32)
            lnse_a = single.tile([P, G], f32)
            negss_a = single.tile([P, G * K], f32)
            nrn_a = single.tile([P, G * K], f32)
            gt = {}
            for g in range(G):
                gt[g] = dict(
                    logits=pg.tile([P, K + 1], f32, tag="logits", name=f"logits{g}"),
                    pdots=pdotsp.tile([P, P], f32, tag="pdots", name=f"pdots{g}"),
                )

            for g in range(G):
                m0 = g * P
                d = gt[g]
                ctx_t = grp.tile([P, C], bf16, tag="ctx")
                pos_t = grp.tile([P, C], bf16, tag="pos")
                nc.sync.dma_start(out=ctx_t[:], in_=ctxg[m0 : m0 + P, :])
                nc.sync.dma_start(out=pos_t[:], in_=posg[m0 : m0 + P, :])

                nc.vector.scalar_tensor_tensor(
                    out=dummy[:], in0=ctx_t[:], scalar=1.0, in1=ctx_t[:],
                    op0=Alu.mult, op1=Alu.mult, accum_out=css_a[:, g : g + 1],
                )
                nc.vector.scalar_tensor_tensor(
                    out=dummy[:], in0=pos_t[:], scalar=1.0, in1=pos_t[:],
                    op0=Alu.mult, op1=Alu.mult, accum_out=pss_a[:, g : g + 1],
                )
                nc.vector.scalar_tensor_tensor(
                    out=dummy[:], in0=ctx_t[:], scalar=1.0, in1=pos_t[:],
                    op0=Alu.mult, op1=Alu.mult, accum_out=cpd_a[:, g : g + 1],
                )

                # ---- sumsq: ScalarE squares (big op), VectorE folds + 3D reduce ----
                for t in range(NKC):
                    sq = sqp.tile([P, KCH, C], bf16, tag="sq")
                    if (g, t) in BF_TILES:
                        bidx = BF_TILES.index((g, t))
                        ntb = astream.tile([P, KCH, C], bf16, tag="ntb", name="ntb")
                        nc.sync.dma_start(out=ntb[:], in_=negab[bidx])
                        nc.vector.tensor_tensor(
                            out=sq[:], in0=ntb[:], in1=ntb[:], op=Alu.mult
                        )
                    elif g == 0 and t < 2:
                        nc.scalar.activation(
                            out=sq[:], in_=nt_pre[t][:], func=Act.Square
                        )
                    else:
                        nt = astream.tile([P, KCH, C], fp8, tag="nt")
                        nc.sync.dma_start(
                            out=nt[:],
                            in_=nega[m0 : m0 + P, t * KCH : (t + 1) * KCH, :],
                        )
                        nc.scalar.activation(out=sq[:], in_=nt[:], func=Act.Square)
                    f1 = fold1p.tile([P, KCH, C // 2], bf16, tag="f1")
                    nc.vector.tensor_tensor(
                        out=f1[:], in0=sq[:, :, 0 : C // 2],
                        in1=sq[:, :, C // 2 : C], op=Alu.add,
                    )
                    f2 = fold2p.tile([P, KCH, C // 4], bf16, tag="f2")
                    nc.vector.tensor_tensor(
                        out=f2[:], in0=f1[:, :, 0 : C // 4],
                        in1=f1[:, :, C // 4 : C // 2], op=Alu.add,
                    )
                    f3 = fold3p.tile([P, KCH, C // 8], bf16, tag="f3")
                    nc.vector.tensor_tensor(
                        out=f3[:], in0=f2[:, :, 0 : C // 8],
                        in1=f2[:, :, C // 8 : C // 4], op=Alu.add,
                    )
                    f4 = fold4p.tile([P, KCH, C // 16], bf16, tag="f4")
                    nc.vector.tensor_tensor(
                        out=f4[:], in0=f3[:, :, 0 : C // 16],
                        in1=f3[:, :, C // 16 : C // 8], op=Alu.add,
                    )
                    f5 = fold5p.tile([P, KCH, C // 32], bf16, tag="f5")
                    nc.vector.tensor_tensor(
                        out=f5[:], in0=f4[:, :, 0 : C // 32],
                        in1=f4[:, :, C // 32 : C // 16], op=Alu.add,
                    )
                    nc.vector.tensor_reduce(
                        out=negss_a[:, g * K + t * KCH : g * K + (t + 1) * KCH],
                        in_=f5[:], axis=mybir.AxisListType.X, op=Alu.add,
                    )

                # ---- dots: TensorE batched mat-vecs into pdt (dots^T) ----
                pdt = pdtp.tile([P, P], f32, tag="pdt")
                for half in range(NMB):
                    mh = m0 + half * MB
                    nts = {}
                    for ch, pool in ((0, ntp0), (1, ntp1)):
                        nts[ch] = pool.tile(
                            [P, MB * K + PAD], fp8, tag=f"nt{ch}", name=f"ntt{ch}"
                        )
                        nc.sync.dma_start(
                            out=nts[ch][:],
                            in_=negt[ch, :, mh * K : mh * K + MB * K + PAD],
                        )
                    for i in range(MB):
                        mcol = half * MB + i
                        mg = m0 + mcol
                        for ch in range(2):
                            nc.tensor.matmul(
                                out=pdt[:, mcol : mcol + 1],
                                lhsT=nts[ch][:, i * K : i * K + P],
                                rhs=ctxt_s[ch][:, mg : mg + 1],
                                start=(ch == 0),
                                stop=(ch == 1),
                            )
                # dots^T [k, m] -> SBUF -> PE transpose -> pdots [m, k]
                sdt = sdtp.tile([P, P], f32, tag="sdt")
                nc.vector.tensor_copy(sdt[:], pdt[:])
                nc.tensor.transpose(d["pdots"][:], sdt[:], identity[:])

            # ---- batched epilogue, grouped by ACT function ----
            nc.scalar.sqrt(css_a[:], css_a[:])
            nc.scalar.sqrt(pss_a[:], pss_a[:])
            nc.scalar.sqrt(negss_a[:], negss_a[:])
            nc.vector.tensor_scalar_max(css_a[:], css_a[:], EPS)
            nc.vector.tensor_scalar_max(pss_a[:], pss_a[:], EPS)
            nc.vector.reciprocal(crn_a[:], css_a[:])
            nc.vector.reciprocal(prn_a[:], pss_a[:])
            nc.vector.tensor_scalar_max(negss_a[:], negss_a[:], EPS)
            nc.vector.reciprocal(nrn_a[:], negss_a[:])
            for g in range(G):
                d = gt[g]
                # logits: col 0 = positive sim, cols 1..K = negative sims
                nc.vector.scalar_tensor_tensor(
                    out=d["logits"][:, 0:1], in0=cpd_a[:, g : g + 1],
                    scalar=crn_a[:, g : g + 1], in1=prn_a[:, g : g + 1],
                    op0=Alu.mult, op1=Alu.mult,
                )
                nc.vector.scalar_tensor_tensor(
                    out=d["logits"][:, 1 : K + 1], in0=d["pdots"][:, 0:K],
                    scalar=crn_a[:, g : g + 1], in1=nrn_a[:, g * K : (g + 1) * K],
                    op0=Alu.mult, op1=Alu.mult,
                )
            # |logits| <= 1 so exp(logit/TEMP) <= e^10 — no max-shift needed
            for g in range(G):
                d = gt[g]
                esc = scrp.tile([P, K + 1], f32, tag="esc")
                nc.scalar.activation(
                    out=esc[:], in_=d["logits"][:], func=Act.Exp,
                    scale=1.0 / TEMP, accum_out=se_a[:, g : g + 1],
                )
            nc.scalar.activation(out=lnse_a[:], in_=se_a[:], func=Act.Ln)
            for g in range(G):
                d = gt[g]
                nc.vector.scalar_tensor_tensor(
                    out=out_t[:, g : g + 1], in0=d["logits"][:, 0:1],
                    scalar=-1.0 / TEMP, in1=lnse_a[:, g : g + 1],
                    op0=Alu.mult, op1=Alu.add,
                )
            nc.sync.dma_start(out=rowloss[:], in_=out_t[:])
    nc.finalize()
    return nc


def _get_nc():
    global _NC
    if _NC is None:
        _NC = _build_nc()
    return _NC


def make_in_maps(context, positive, negatives, mask_indices):
    import ml_dtypes

    bf = ml_dtypes.bfloat16
    f8 = ml_dtypes.float8_e4m3
    context = np.asarray(context, dtype=np.float32)
    positive = np.asarray(positive, dtype=np.float32)
    negatives = np.asarray(negatives, dtype=np.float32)
    mask = np.asarray(mask_indices).astype(bool)
    in_maps = []
    for b in range(B):
        idx = np.flatnonzero(mask[b])
        assert idx.size == M, f"row {b}: expected {M} masked, got {idx.size}"
        ctx_m = np.ascontiguousarray(context[b].T[idx])  # [M, C] f32
        pos_m = np.ascontiguousarray(positive[b].T[idx])  # [M, C] f32
        neg = negatives[b]  # [M, K, C] f32
        # c-major transposed negatives for the PE, flat (m,k) + PAD zeros
        negT = neg.transpose(2, 0, 1).reshape(C, M * K).astype(f8)  # [C, M*K]
        negt = np.zeros((2, P, M * K + PAD), dtype=f8)
        negt[0, :, : M * K] = negT[:P]
        negt[1, :, : M * K] = negT[P:]
        ctxT = ctx_m.T.astype(f8)  # [C, M]
        ctxt = np.stack([ctxT[:P], ctxT[P:]])  # [2, 128, M]
        negab = np.stack(
            [
                neg[g * P : (g + 1) * P, t * KCH : (t + 1) * KCH, :].astype(bf)
                for (g, t) in BF_TILES
            ]
        )
        in_maps.append(
            {
                "nega": neg.astype(f8),
                "negab": negab,
                "negt": negt,
                "ctxt": np.ascontiguousarray(ctxt),
                "ctxg": ctx_m.astype(bf),
                "posg": pos_m.astype(bf),
            }
        )
    return in_maps


def kernel(context, positive, negatives, mask_indices, num_masked):
    from concourse.bass_utils import run_bass_kernel_spmd

    nm = int(np.asarray(num_masked))
    assert nm == M, f"kernel hardcodes num_masked={M}, got {nm}"
    assert np.asarray(context).shape == (B, C, T)
    assert np.asarray(negatives).shape == (B, M, K, C)

    in_maps = make_in_maps(context, positive, negatives, mask_indices)
    res = run_bass_kernel_spmd(_get_nc(), in_maps, core_ids=list(range(B)))
    total = np.float64(0.0)
    for r in res.results:
        total += r["rowloss"].astype(np.float64).sum()
    return np.float32(total / (B * M))
